# revision 45
# baseline (speedup 1.0000x reference)
"""LSTMCell on 8 Trainium2 NeuronCores, data-parallel over the batch.

Full inputs: x/h_t/c_t [65536,128] f32, 8 gate weight matrices [128,128],
4 biases [128]. Returns (h_new, c_new) as [65536,128] f32 each.

Design (v19, ~55.9us, from v13's ~59.6us; fp16 matmul path, transposed
layout, host-side transposes; steady state is ACT(sigmoid)-bound):
  - Host transposes x/h/c per core to [128 feat, 8192 batch] fp16 and
    pre-concats weights as WxT/WhT [128 in, 512 gate-rows] fp16 in gate
    order [o, i, f, 2*g] (g prescaled by 2 for the tanh-via-sigmoid
    trick).  fp16 operands: bf16 rounding through the gates was the
    dominant error term.
  - Per batch group of 512 cols: 8 matmuls (weights stationary)
    accumulate gates^T into a 4-bank PSUM quad [128,2048]; ONE sigmoid
    per quad -> bf16 SBUF sig2 tile (2 groups per pair tile).
  - DVE chain per pair: gt=2s-1 [tensor_scalar], ig=i*gt, fc=f*c,
    c'=ig+fc (all 2x-mode 16-bit stock ops, ~0.52ns/col).
  - tanh(c') is SPLIT between engines to balance ACT vs DVE (~37us
    each): pairs 0-3 use two injected custom fused DVE ops
    (LSTM_TANH_RECIP: r=1/(c'^2+TD) via BITWISE_NOT exponent-flip seed
    + one Newton step; LSTM_TANH_ZMUL: z=clamp((r*TC2+TC1)*c', +-1),
    minimax-fitted to |z-tanh|<=3.7e-3, ~1.04ns/col); pairs 4-6 use the
    ACT tanh table; the last pair uses per-group ACT tanh (shortest
    tail).  Custom ops are registered per-NEFF via the dve_ops
    extension point (no firmware change).
  - Emission discipline (ALL engine queues are FIFO; a mis-ordered
    dependency head-blocks a queue and can idle the PE >3.4us, which
    re-throttles its HAM clock gate to half rate for the rest of the
    kernel): custom r/z/h' for pair P is emitted one pair later; ACT
    tanh for pair P two pairs later (else the tanh head-blocks the
    sigma FIFO waiting for c'); T(NP-3) is emitted between the last
    pair's two sigmas to keep it off the kernel tail.  hnt chunk DMAs
    fire only once ALL pairs of the chunk have their h' emitted
    (completion order is scrambled by the mixed deferrals).
  - DMA: x/h/c interleaved per-chunk in consumption order on the SINGLE
    sync queue (transfer order = consumption order; any reorder or
    second input queue steals bandwidth from the x/h stream the PE
    blocks on).  Weights + warmup memsets on the gpsimd queue (never
    issue DMA from the scalar queue - it evicts the ACT table).
    Outputs in 8/4/2/2-group chunks on sync with per-group DMAs at the
    very end.  9 junk warmup matmuls bridge the queue preamble so PE
    HAM activity never lapses (fewer => half-clock PE, measured 75us).
  - Error budget: 9.1e-3 vs fp64 reference (limit 2e-2): ~7.5e-3 fp16
    gate path (as v13) + ~3.7e-3 rational-tanh model error on the
    custom pairs.
"""
import numpy as np
import ml_dtypes
from contextlib import ExitStack

import concourse.bass as bass
import concourse.tile as tile
from concourse import bacc, mybir
from concourse.bass_utils import run_bass_kernel_spmd

from concourse import dve_ops as _dop
from concourse.dve_spec import (
    Spec, Src0, Src1, C0, C1, C2, Bin, AluOp as DAlu, maxx, minn, sq,
    lower as _dve_lower, _has_src1,
)
from concourse.dve_uop import DveOpSpec

F32 = mybir.dt.float32
F16 = mybir.dt.float16
BF16 = mybir.dt.bfloat16
NPBF = ml_dtypes.bfloat16
AF = mybir.ActivationFunctionType
ALU = mybir.AluOpType

NCORES = 8
BC = 8192            # batch rows per core
GW = 512             # batch columns per group (one PSUM bank)
NG = BC // GW        # 16 groups
H = 128              # hidden size
# input chunks in groups: small (fast fill), then growing
ICHUNKS = [(0, 1), (1, 1), (2, 2), (4, 4), (8, 8)]
# output chunks (start group, n groups): big, medium, small tail
OCHUNKS = [(0, 8), (8, 4), (12, 2), (14, 2)]

# --- custom fused DVE ops: rational tanh(c') -------------------------------
# r = 1/(c'^2+TD) via BITWISE_NOT exponent-flip seed + one Newton step
# (fp32 internally, fp16 in/out); z = clamp((r*TC2 + TC1)*c', -1, 1).
# Joint minimax fit over the fp16 path: max |z - tanh| ~= 3.7e-3.
TR0, TR1 = -0.23549792, 2.0017324
TC1, TC2, TD = 0.12762096, 2.40399202, 2.78807243
# pairs whose tanh(c') runs on the custom DVE path (ACT keeps the rest;
# blending equalizes ACT ~4.4us/pair vs DVE ~4.2us/pair)
CUSTOM_TANH_PAIRS = (0, 1, 2, 3)


def _register_dve_op(name, spec, subdim=False):
    if name in _dop._SUB_OPCODE_FOR_NAME:
        return next(op for op in _dop.OPS if op.name == name)
    row = _dop._CUSTOM_DVE_ROW_BASE + len(_dop.OPS)
    assert row < 0x20, "custom DVE row overflow"
    shas = {}
    for ver in ("v3", "v4"):
        try:
            tmp = DveOpSpec(name=name, opcode=row,
                            uops=_dve_lower(spec, ver=ver),
                            rd1_en=_has_src1(spec))
            shas[ver] = tmp.sha(ver)
        except Exception:
            pass
    op = _dop.DveOp(name, spec, subdim, shas)
    _dop.OPS.append(op)
    _dop._SUB_OPCODE_FOR_NAME[name] = row
    _dop.CUSTOM_DVE_SPECS[name] = spec
    return op


def _ref_tanh_recip(in0, in1, c0, c1, c2):
    d = (in0.astype(np.float32) ** 2 + np.float32(c0)).astype(np.float32)
    nd = (~d.view(np.int32)).view(np.float32)
    y0 = nd * np.float32(c1)
    return y0 * (np.float32(c2) - d * y0)


def _ref_tanh_zmul(in0, in1, c0, c1, c2):
    t = ((in0.astype(np.float32) * np.float32(c0) + np.float32(c1))
         * in1.astype(np.float32))
    return np.maximum(np.minimum(t, np.float32(c2)), -np.float32(c2))


_d = sq(Src0) + C0
_nd = Bin(DAlu.BITWISE_NOT, _d, _d)
_y0 = _nd * C1
TANH_RECIP = _register_dve_op(
    "LSTM_TANH_RECIP", Spec(body=_y0 * (C2 - _d * _y0),
                            reference=_ref_tanh_recip))
_t = (Src0 * C0 + C1) * Src1
TANH_ZMUL = _register_dve_op(
    "LSTM_TANH_ZMUL", Spec(body=maxx(minn(_t, C2), -C2),
                           reference=_ref_tanh_zmul))

_CACHE = {}


def _build(has_bias: bool):
    nc = bacc.Bacc("TRN2", target_bir_lowering=False, debug=False)
    xt = nc.dram_tensor("xt", [H, BC], F16, kind="ExternalInput").ap()
    ht = nc.dram_tensor("ht", [H, BC], F16, kind="ExternalInput").ap()
    ct = nc.dram_tensor("ct", [H, BC], F16, kind="ExternalInput").ap()
    wxt = nc.dram_tensor("wxt", [H, 4 * H], F16, kind="ExternalInput").ap()
    wht = nc.dram_tensor("wht", [H, 4 * H], F16, kind="ExternalInput").ap()
    if has_bias:
        bias = nc.dram_tensor("bias", [H, 4], F32, kind="ExternalInput").ap()
    hnt = nc.dram_tensor("hnt", [H, BC], F16, kind="ExternalOutput").ap()
    cnt = nc.dram_tensor("cnt", [H, BC], F16, kind="ExternalOutput").ap()


    with tile.TileContext(nc) as tc:
        with ExitStack() as ctx:
            const = ctx.enter_context(tc.tile_pool(name="const", bufs=1))
            ina = ctx.enter_context(tc.tile_pool(name="ina", bufs=1))
            qp = ctx.enter_context(tc.tile_pool(name="qp", bufs=2, space="PSUM"))
            tp = ctx.enter_context(tc.tile_pool(name="tp", bufs=3))
            sp = ctx.enter_context(tc.tile_pool(name="sp", bufs=7))
            op = ctx.enter_context(tc.tile_pool(name="op", bufs=3))

            # Input tiles in chunks per tensor: small chunk first for fast
            # pipeline fill, then medium/large for DMA efficiency.
            xts, hts, cts = [], [], []
            for ci, (cs, cw) in enumerate(ICHUNKS):
                for lst, nm in ((xts, "x"), (hts, "h"), (cts, "c")):
                    lst.append(ina.tile([H, cw * GW], F16,
                                        name=f"{nm}{ci}"))
            def cbounds(ci):
                cs, cw = ICHUNKS[ci]
                return cs * GW, (cs + cw) * GW
            # gpsimd queue first: warmup/dummy memsets (so the PE warmups
            # are not blocked behind any queue preamble), then the weights.
            # NOTE: never issue DMA from the scalar queue - HWDGE on the
            # Activation engine evicts its ACT table (~1.3us reload).
            junk = const.tile([H, GW], F16)
            nc.gpsimd.memset(junk[:], 0.0)
            dummy = const.tile([H, 8], F32)
            nc.gpsimd.memset(dummy[:], 0.0)
            wx_sb = const.tile([H, 4 * H], F16)
            nc.gpsimd.dma_start(wx_sb[:], wxt)
            wh_sb = const.tile([H, 4 * H], F16)
            nc.gpsimd.dma_start(wh_sb[:], wht)
            if has_bias:
                b_sb = const.tile([H, 4], F32)
                nc.gpsimd.dma_start(b_sb[:], bias)
            # first x/h chunk also from gpsimd: its queue preamble ends
            # ~1us before sync's, so quad 0's data lands earlier
            # x/h chunks 0-3 gate matmuls -> issue them first on the sync
            # queue; c trails (consumed later by the chain).  One queue =>
            # transfer order matches consumption order.
            nchunk = len(ICHUNKS)
            for ci in range(nchunk):
                c0, c1 = cbounds(ci)
                nc.sync.dma_start(xts[ci][:], xt[:, c0:c1])
                nc.sync.dma_start(hts[ci][:], ht[:, c0:c1])
                nc.sync.dma_start(cts[ci][:], ct[:, c0:c1])

            # ACT table preload (sigmoid set includes tanh) overlaps DMA fill
            dummy2 = const.tile([H, 8], F32)
            nc.scalar.activation(dummy2[:], dummy[:], AF.Sigmoid)

            def in_slice(tiles, g, w):
                c0 = g * GW
                for ci, (cs, cw) in enumerate(ICHUNKS):
                    if c0 + w <= (cs + cw) * GW:
                        return tiles[ci][:, c0 - cs * GW:c0 - cs * GW + w]
                raise AssertionError("slice straddles input chunks")

            # HAM warmup on the junk tile while DMAs stream; >=8 needed:
            # HAM un-throttle wants ~3.4us of SUSTAINED PE activity, else
            # the PE runs at half clock for the WHOLE kernel
            warm = qp.tile([H, 2048], F32, name="warm", tag="quad")
            for _ in range(9):
                nc.tensor.matmul(warm[:, 0:GW], junk[:, 0:H], junk[:],
                                 start=True, stop=True)

            NP = NG // 2  # pairs
            sig2s = {}

            # pair -> (chunk_start_group, chunk_width, local_offset, is_last)
            pair_chunk = {}
            for cs, cw in OCHUNKS:
                for g in range(cs, cs + cw, 2):
                    pair_chunk[g // 2] = (cs, cw * GW, (g - cs) * GW,
                                          g + 2 == cs + cw)

            def emit_tanh_h2(Pa):
                """tanh + h' for pairs (Pa, Pa+1) in one ACT pass."""
                Pb = Pa + 1
                cs, cw, lo_a, _ = pair_chunk[Pa]
                cnb, hnb = cn_hn[Pa]
                tcp = tp.tile([H, 2048], BF16, name=f"tc{Pa}", tag="tc")
                nc.scalar.activation(tcp[:], cnb[:, lo_a:lo_a + 4 * GW],
                                     AF.Tanh)
                for j, P in enumerate((Pa, Pb)):
                    lo = pair_chunk[P][2]
                    last = pair_chunk[P][3]
                    sig2 = sig2s.pop(P)
                    o3 = sig2[:].rearrange("p (t x) -> p t x",
                                           t=2)[:, :, 0:512]
                    h3 = hnb[:, lo:lo + 2 * GW].rearrange(
                        "p (t x) -> p t x", t=2)
                    t3 = tcp[:, j * 1024:(j + 1) * 1024].rearrange(
                        "p (t x) -> p t x", t=2)
                    nc.vector.tensor_mul(h3, o3, t3)
                    if last:
                        nc.sync.dma_start(hnt[:, cs * GW:cs * GW + cw],
                                          hnb[:])

            def emit_tanh_h(P):
                """tanh + h' + (maybe) hn DMA for pair P (c' already done)."""
                cs, cw, lo, last = pair_chunk[P]
                cnb, hnb = cn_hn[P]
                tcp = tp.tile([H, 1024], BF16, name=f"tc{P}", tag="tc")
                nc.scalar.activation(tcp[:], cnb[:, lo:lo + 2 * GW], AF.Tanh)
                sig2 = sig2s.pop(P)
                o3 = sig2[:].rearrange("p (t x) -> p t x", t=2)[:, :, 0:512]
                h3 = hnb[:, lo:lo + 2 * GW].rearrange("p (t x) -> p t x", t=2)
                t3 = tcp[:].rearrange("p (t x) -> p t x", t=2)
                nc.vector.tensor_mul(h3, o3, t3)
                mark_h_done(P)

            hdone = set()

            def mark_h_done(P):
                """Fire an hnt chunk DMA only once ALL its pairs' h' writes
                have been emitted - pair completion order is scrambled by
                the mixed 1-pair (custom) / 2-pair (ACT tanh) deferrals."""
                hdone.add(P)
                cs, cw, _, _ = pair_chunk[P]
                pairs = [g // 2 for g in range(cs, cs + cw // GW, 2)]
                if cs + cw // GW == NG:
                    return  # tail chunk: per-group DMAs happen in the tail
                if all(q in hdone for q in pairs):
                    nc.sync.dma_start(hnt[:, cs * GW:cs * GW + cw],
                                      cn_hn[pairs[0]][1][:])

            def emit_custom_h(P):
                """custom r/z (rational tanh) + h' + (maybe) hn DMA for
                pair P (c' already done).  Deferred one pair so this ~3us
                of DVE work covers the pool fc latency of the next pair."""
                cs, cw, lo, last = pair_chunk[P]
                cnb, hnb = cn_hn[P]
                cpr = cnb[:, lo:lo + 2 * GW]
                rt = tp.tile([H, 2 * GW], F16, name=f"r{P}", tag="rt")
                nc.vector._custom_dve(TANH_RECIP, out=rt[:], in0=cpr,
                                      s0=TD, s1=TR0, imm2=TR1)
                zt = tp.tile([H, 2 * GW], F16, name=f"z{P}", tag="zt")
                nc.vector._custom_dve(TANH_ZMUL, out=zt[:], in0=rt[:],
                                      in1=cpr, s0=TC2, s1=TC1, imm2=1.0)
                sig2c = sig2s.pop(P)
                o3 = sig2c[:].rearrange("p (t x) -> p t x",
                                        t=2)[:, :, 0:512]
                h3 = hnb[:, lo:lo + 2 * GW].rearrange(
                    "p (t x) -> p t x", t=2)
                z3 = zt[:].rearrange("p (t x) -> p t x", t=2)
                nc.vector.tensor_mul(h3, o3, z3)
                mark_h_done(P)

            cn_hn = {}
            cn_buf = hn_buf = None
            for P in range(NP):
                g0 = 2 * P
                cs, cw, lo, last = pair_chunk[P]
                if lo == 0:
                    cn_buf = op.tile([H, cw], F16, name=f"cn{g0}", tag="cn")
                    hn_buf = op.tile([H, cw], F16, name=f"hn{g0}", tag="hn")
                cn_hn[P] = (cn_buf, hn_buf)
                sig2 = sp.tile([H, 4096], BF16, name=f"s{P}", tag="sig")
                sig2s[P] = sig2

                def emit_dve(g_first, ng, tag_sfx):
                    """c'-chain for ng groups starting at g_first (pair P).
                    ig/fc/c' are fp16: bf16 rounding of the large ig/fc
                    terms would dominate the error after cancellation."""
                    w = ng * GW
                    gg = g_first - g0

                    def sl(bank):
                        s = sig2[:].rearrange("p (t x) -> p t x", t=2)
                        s = s[:, gg:gg + ng, bank * GW:(bank + 1) * GW]
                        return s

                    def r3(ap2d):
                        return ap2d.rearrange("p (t x) -> p t x", t=ng)

                    c3 = r3(in_slice(cts, g_first, w))
                    gt = tp.tile([H, w], BF16, name=f"gt{tag_sfx}", tag="gt")
                    nc.vector.tensor_scalar(r3(gt[:]), sl(0 + 3), 2.0, 1.0,
                                            ALU.mult, ALU.subtract)
                    ig = tp.tile([H, w], F16, name=f"ig{tag_sfx}", tag="ig")
                    nc.vector.tensor_mul(r3(ig[:]), sl(1), r3(gt[:]))
                    fc = tp.tile([H, w], F16, name=f"fc{tag_sfx}", tag="fc")
                    nc.vector.tensor_mul(r3(fc[:]), sl(2), c3)
                    lg = lo + gg * GW
                    nc.vector.tensor_add(cn_buf[:, lg:lg + w], ig[:], fc[:])
                    if last and gg + ng == 2:
                        nc.sync.dma_start(
                            cnt[:, cs * GW:cs * GW + cw], cn_buf[:])

                lastP = P == NP - 1
                for gg in range(2):
                    g = g0 + gg
                    xs = in_slice(xts, g, GW)
                    hs = in_slice(hts, g, GW)
                    split = (lastP or P == 0) and not has_bias
                    quad = qp.tile([H, 2048], F32, name=f"q{g}", tag="quad")
                    so = sig2[:, gg * 2048:(gg + 1) * 2048]
                    for k in ([1, 2, 3, 0] if split else range(4)):
                        nc.tensor.matmul(quad[:, k * GW:(k + 1) * GW],
                                         wx_sb[:, k * H:(k + 1) * H], xs,
                                         start=True, stop=False)
                        nc.tensor.matmul(quad[:, k * GW:(k + 1) * GW],
                                         wh_sb[:, k * H:(k + 1) * H], hs,
                                         start=False, stop=True)
                    if has_bias:
                        for k in range(4):
                            nc.scalar.activation(
                                so[:, k * GW:(k + 1) * GW],
                                quad[:, k * GW:(k + 1) * GW],
                                AF.Sigmoid, bias=b_sb[:, k:k + 1])
                    elif split:
                        # i/f/s banks first: unblocks the DVE chain; the
                        # o bank (only needed by h') trails
                        nc.scalar.activation(so[:, GW:], quad[:, GW:],
                                             AF.Sigmoid)
                        nc.scalar.activation(so[:, 0:GW], quad[:, 0:GW],
                                             AF.Sigmoid)
                    else:
                        nc.scalar.activation(so, quad[:], AF.Sigmoid)
                    if lastP or P == 0:
                        # per-group chain: shortens tail (last pair) and
                        # avoids straddling input chunks (first pair)
                        emit_dve(g, 1, f"p{P}g{gg}")
                    if gg == 0 and P == NP - 1 and \
                            (P - 2) not in CUSTOM_TANH_PAIRS:
                        # pull T(NP-3) off the kernel tail: emit it between
                        # the last pair's two sigmas instead of after them
                        emit_tanh_h(P - 2)
                    if gg == 1:
                        # custom r/z/h' deferred ONE pair (covers fc
                        # latency); ACT tanh deferred TWO pairs, else the
                        # tanh head-blocks the sigma FIFO waiting for c'
                        # (measured 3-5us ACT stalls per tanh)
                        if P >= 1 and (P - 1) in CUSTOM_TANH_PAIRS \
                                and (P - 1) != NP - 1:
                            emit_custom_h(P - 1)
                        if P >= 2 and (P - 2) not in CUSTOM_TANH_PAIRS \
                                and P != NP - 1:
                            emit_tanh_h(P - 2)
                        if P == NP - 1 and \
                                (P - 1) not in CUSTOM_TANH_PAIRS:
                            emit_tanh_h(P - 1)

                if not (lastP or P == 0):
                    emit_dve(g0, 2, f"p{P}")

            # last pair: per-group ACT tanh/h'/hn (ACT is idle after the
            # last sigma, so this is the shortest tail)
            P = NP - 1
            cs, cw, lo, _ = pair_chunk[P]
            cnb, hnb = cn_hn[P]
            sig2 = sig2s.pop(P)
            for gg in range(2):
                lg = lo + gg * GW
                tcg = tp.tile([H, GW], BF16, name=f"tcz{gg}", tag="tc")
                nc.scalar.activation(tcg[:], cnb[:, lg:lg + GW], AF.Tanh)
                o2 = sig2[:, gg * 2048:gg * 2048 + 512]
                nc.vector.tensor_mul(hnb[:, lg:lg + GW], o2, tcg[:])
                gcol = (cs + gg * (cw // GW - 1)) * GW
                nc.sync.dma_start(hnt[:, gcol:gcol + GW],
                                  hnb[:, lg:lg + GW])
    nc.compile()
    return nc


def _run(inputs, trace=False, tmpdir=None):
    x = np.asarray(inputs["x"], dtype=np.float32)
    h = np.asarray(inputs["h_t"], dtype=np.float32)
    c = np.asarray(inputs["c_t"], dtype=np.float32)
    # gate order [i, f, o, g]; W_g/b_g scaled by 2 for the tanh-via-sigmoid
    wx = np.concatenate([inputs["W_io"], inputs["W_ii"], inputs["W_if"],
                         2.0 * np.asarray(inputs["W_ig"])], axis=0)
    wh = np.concatenate([inputs["W_ho"], inputs["W_hi"], inputs["W_hf"],
                         2.0 * np.asarray(inputs["W_hg"])], axis=0)
    b = np.concatenate([inputs["b_o"], inputs["b_i"], inputs["b_f"],
                        2.0 * np.asarray(inputs["b_g"])], axis=0)
    wxt = np.ascontiguousarray(wx.T).astype(np.float16)
    wht = np.ascontiguousarray(wh.T).astype(np.float16)
    has_bias = bool(np.any(b))

    key = has_bias
    if key not in _CACHE:
        _CACHE[key] = _build(has_bias)
    nc = _CACHE[key]

    x16 = x.astype(np.float16)
    h16 = h.astype(np.float16)
    c16 = c.astype(np.float16)
    in_maps = []
    for i in range(NCORES):
        s = slice(i * BC, (i + 1) * BC)
        m = {
            "xt": np.ascontiguousarray(x16[s].T),
            "ht": np.ascontiguousarray(h16[s].T),
            "ct": np.ascontiguousarray(c16[s].T),
            "wxt": wxt,
            "wht": wht,
        }
        if has_bias:
            m["bias"] = np.ascontiguousarray(
                b.reshape(4, H).T.astype(np.float32))
        in_maps.append(m)

    res = run_bass_kernel_spmd(nc, in_maps, core_ids=list(range(NCORES)),
                               trace=trace, tmpdir=tmpdir)
    h_new = np.empty((NCORES * BC, H), dtype=np.float32)
    c_new = np.empty((NCORES * BC, H), dtype=np.float32)
    for i, r in enumerate(res.results):
        s = slice(i * BC, (i + 1) * BC)
        h_new[s] = r["hnt"].T
        c_new[s] = r["cnt"].T
    return h_new, c_new, res


def kernel(**inputs):
    h_new, c_new, _ = _run(inputs, trace=False)
    return h_new, c_new

